# revision 13
# baseline (speedup 1.0000x reference)
"""DetectorLoss on 8 Trainium2 NeuronCores (Bass/Tile).

Strategy (data-parallel over batch, 4 images per core):
  * Host interleaves per-cell records [d0..d3, obj, cls0..cls19, pad]
    (32 f32 = 128B) into one cell-major DRAM region, so the per-positive
    gather needs only K=4 indirect_dma_start instructions (the proven
    one-offset-per-partition HW shape, 128B contiguous per record) vs 12
    per-value gathers in the baseline.  The ground-truth class value is
    selected on-chip with a host-built one-hot mask + tensor_reduce.
  * pred_obj background streamed as bf16 (614KB/core): DVE computes
    y = clamp(x,-1,1) and p = x*y in 2x packed mode; ACT accumulates
    sum(p) (Identity+accum) and sum(y^2) (Square+accum) per chunk:
      sum sl1 = sum(x*y) - 0.5*sum(y*y)
  * One manual LoadActFuncSet(6) pins the {exp,ln,square,identity}
    table: zero mid-kernel table reloads (baseline had 4 x 1.28us).
  * Per-positive SIoU/obj/cls math on DVE; unary tails (1-e)^4 on ACT.
  * Host combines per-core partial sums (weighted means).
"""
import numpy as np

B, A, C, H, W = 32, 3, 20, 160, 160
HW = H * W
M = 8            # cores
Bm = B // M      # images per core
NCELL = Bm * A * HW           # 307200 cells per core
SZ_OBJ = NCELL
REC = 32                      # f32 per cell record
NCH = 2                       # pred_obj stream chunks
CW_ = (SZ_OBJ // 128) // NCH  # 1200 cols per chunk
NF = 11                       # host-data planes
# partial cols: per chunk {sum x*y, sum y*y}; then iou, -ln*valid,
# t1*win, s2m*win
NCOLS = 2 * NCH + 4

_NC_CACHE = {}


def _build_nc(K):
    import itertools

    import concourse.bass as bass
    import concourse.bacc as bacc
    import concourse.tile as tile
    from concourse import mybir

    f32 = mybir.dt.float32
    bf16 = mybir.dt.bfloat16
    i32 = mybir.dt.int32
    op = mybir.AluOpType
    act = mybir.ActivationFunctionType
    # hdr cols (int32): K rec offsets | 11K hd planes | 20K one-hot mask
    HC = (1 + NF + C) * K

    nc = bacc.Bacc("TRN2", target_bir_lowering=False, debug=False)
    rec_p = nc.dram_tensor("rec", [NCELL * REC, 1], f32, kind="ExternalInput")
    objh_p = nc.dram_tensor("objh", [SZ_OBJ, 1], bf16, kind="ExternalInput")
    hdr_p = nc.dram_tensor("hdr", [128, HC], i32, kind="ExternalInput")
    out_p = nc.dram_tensor("partials", [128, NCOLS], f32, kind="ExternalOutput")

    with tile.TileContext(nc) as tc, \
         tc.tile_pool(name="io", bufs=1) as io, \
         tc.tile_pool(name="wk", bufs=1) as wk, \
         tc.tile_pool(name="st", bufs=2) as st:
        # pin the activation table once: set 6 covers exp/ln/square/identity
        ld = mybir.InstLoadActFuncSet(
            name=nc.get_next_instruction_name(), ins=[], outs=[],
            act_func_set_id=6)
        nc.scalar.add_instruction(ld)

        hdr = io.tile([128, HC], i32)
        nc.sync.dma_start(out=hdr[:, :], in_=hdr_p[:, :])
        partials = io.tile([128, NCOLS], f32)

        tt = nc.vector.tensor_tensor
        ts = nc.vector.tensor_scalar
        stt = nc.vector.scalar_tensor_tensor
        A_ = nc.scalar.activation

        def hdf(a, b):
            return hdr[:, (1 + a) * K:(1 + b) * K].bitcast(f32)

        pxy = hdf(0, 2)
        anc = hdf(2, 4)
        gtc = hdf(4, 6)
        gtwh = hdf(6, 8)
        cnt_ = hdf(8, 9)
        win = hdf(9, 10)
        valid = hdf(10, 11)
        maskv = hdr[:, (1 + NF) * K:(1 + NF + C) * K].bitcast(f32)

        PR = [128, 2 * K]
        SG = [128, K]
        _cnt = itertools.count()

        def pr():
            n = f"pr{next(_cnt)}"
            return wk.tile(PR, f32, name=n, tag=n)

        def sg():
            n = f"sg{next(_cnt)}"
            return wk.tile(SG, f32, name=n, tag=n)

        def lo(t):
            return t[:, 0:K]

        def hi(t):
            return t[:, K:2 * K]

        # ---- record gathers: K instructions, 128B contiguous per record --
        G = wk.tile([128, K, REC], f32, name="G", tag="G")
        for i in range(K):
            nc.gpsimd.indirect_dma_start(
                out=G[:, i, :], out_offset=None, in_=rec_p[:, :],
                in_offset=bass.IndirectOffsetOnAxis(ap=hdr[:, i:i + 1],
                                                    axis=0))

        # ---- pre-gather work (only needs hdr) ----
        b2lo = pr(); stt(out=b2lo[:], in0=gtwh, scalar=-0.5, in1=gtc,
                         op0=op.mult, op1=op.add)
        b2hi = pr(); stt(out=b2hi[:], in0=gtwh, scalar=0.5, in1=gtc,
                         op0=op.mult, op1=op.add)
        area2 = sg(); tt(out=area2[:], in0=gtwh[:, 0:K], in1=gtwh[:, K:2 * K],
                         op=op.mult)
        rc = sg(); nc.vector.reciprocal(out=rc[:], in_=cnt_)

        # ---- streamed background over pred_obj (bf16):
        # DVE: y = clamp(x,-1,1) (2x), p = x*y (2x);
        # ACT: sum(p) via Identity+accum, sum(y^2) via Square+accum
        xs, ys, ps = [], [], []
        for c in range(NCH):
            x = st.tile([128, CW_], bf16)
            chunk = bass.AP(tensor=objh_p[:, :].tensor, offset=c * 128 * CW_,
                            ap=[[CW_, 128], [1, CW_]])
            nc.sync.dma_start(out=x[:, :], in_=chunk)
            y = st.tile([128, CW_], bf16)
            ts(out=y[:, :], in0=x[:, :], scalar1=1.0, scalar2=-1.0,
               op0=op.min, op1=op.max)
            p = st.tile([128, CW_], bf16)
            tt(out=p[:, :], in0=x[:, :], in1=y[:, :], op=op.mult)
            xs.append(x); ys.append(y); ps.append(p)

        # strided plane-major views of the gathered delta-box values:
        # d01 = [d0-plane | d1-plane], d23 = [d2-plane | d3-plane]
        dv = G[:, :, :]
        d01 = bass.AP(tensor=dv.tensor, offset=dv.offset,
                      ap=[dv.ap[0], [1, 2], [REC, K]])
        d23 = bass.AP(tensor=dv.tensor, offset=dv.offset + 2,
                      ap=[dv.ap[0], [1, 2], [REC, K]])
        po = bass.AP(tensor=dv.tensor, offset=dv.offset + 4,
                     ap=[dv.ap[0], [REC, K]])
        gcls = bass.AP(tensor=dv.tensor, offset=dv.offset + 5,
                       ap=[dv.ap[0], [REC, K], [1, C]])

        # ---- ACT queue: bg accums first (ready early), then gather-gated
        for c in range(NCH):
            s1 = st.tile([128, CW_], bf16)
            A_(out=s1[:, :], in_=ps[c][:, :], func=act.Identity,
               accum_out=partials[:, 2 * c:2 * c + 1])
            s2 = st.tile([128, CW_], bf16)
            A_(out=s2[:, :], in_=ys[c][:, :], func=act.Square,
               accum_out=partials[:, 2 * c + 1:2 * c + 2])
        e2 = pr(); A_(out=e2[:], in_=d01, func=act.Exp, scale=2.0)
        ex = pr(); A_(out=ex[:], in_=d23, func=act.Exp)

        # ---- per-positive math (DVE unless noted) ----
        e2p = pr(); ts(out=e2p[:], in0=e2[:], scalar1=1.0, scalar2=None,
                       op0=op.add)
        re2 = pr(); nc.vector.reciprocal(out=re2[:], in_=e2p[:])
        th = pr(); ts(out=th[:], in0=re2[:], scalar1=-2.0, scalar2=1.0,
                      op0=op.mult, op1=op.add)
        c1 = pr(); tt(out=c1[:], in0=th[:], in1=pxy, op=op.add)
        wh1 = pr(); stt(out=wh1[:], in0=ex[:], scalar=float(W), in1=anc,
                        op0=op.mult, op1=op.mult)
        b1lo = pr(); stt(out=b1lo[:], in0=wh1[:], scalar=-0.5, in1=c1[:],
                         op0=op.mult, op1=op.add)
        b1hi = pr(); stt(out=b1hi[:], in0=wh1[:], scalar=0.5, in1=c1[:],
                         op0=op.mult, op1=op.add)
        mnhi = pr(); tt(out=mnhi[:], in0=b1hi[:], in1=b2hi[:], op=op.min)
        mxlo = pr(); tt(out=mxlo[:], in0=b1lo[:], in1=b2lo[:], op=op.max)
        itax = pr(); tt(out=itax[:], in0=mnhi[:], in1=mxlo[:], op=op.subtract)
        itax2 = pr(); ts(out=itax2[:], in0=itax[:], scalar1=0.0, scalar2=None,
                         op0=op.max)
        inter = sg(); tt(out=inter[:], in0=lo(itax2), in1=hi(itax2),
                         op=op.mult)
        area1 = sg(); tt(out=area1[:], in0=lo(wh1), in1=hi(wh1), op=op.mult)
        u1 = sg(); tt(out=u1[:], in0=area1[:], in1=area2[:], op=op.add)
        u2 = sg(); tt(out=u2[:], in0=u1[:], in1=inter[:], op=op.subtract)
        ru = sg(); nc.vector.reciprocal(out=ru[:], in_=u2[:])
        iou = sg(); tt(out=iou[:], in0=inter[:], in1=ru[:], op=op.mult)
        cwmax = pr(); tt(out=cwmax[:], in0=b1hi[:], in1=b2hi[:], op=op.max)
        cwmin = pr(); tt(out=cwmin[:], in0=b1lo[:], in1=b2lo[:], op=op.min)
        cw = pr(); tt(out=cw[:], in0=cwmax[:], in1=cwmin[:], op=op.subtract)
        # s_cw/s_ch = gt_center - pred_center
        s = pr(); tt(out=s[:], in0=gtc, in1=c1[:], op=op.subtract)
        # angle_cost = 2*|s_cw*s_ch| / sigma^2 (sqrt-free)
        sqs = pr(); tt(out=sqs[:], in0=s[:], in1=s[:], op=op.mult)
        sig2 = sg(); tt(out=sig2[:], in0=lo(sqs), in1=hi(sqs), op=op.add)
        prod = sg(); tt(out=prod[:], in0=lo(s), in1=hi(s), op=op.mult)
        aprod = sg(); stt(out=aprod[:], in0=prod[:], scalar=-1.0, in1=prod[:],
                          op0=op.mult, op1=op.max)
        rsig2 = sg(); nc.vector.reciprocal(out=rsig2[:], in_=sig2[:])
        angle = sg(); stt(out=angle[:], in0=aprod[:], scalar=2.0, in1=rsig2[:],
                          op0=op.mult, op1=op.mult)
        gamma = sg(); ts(out=gamma[:], in0=angle[:], scalar1=-2.0, scalar2=None,
                         op0=op.add)
        rcw = pr(); nc.vector.reciprocal(out=rcw[:], in_=cw[:])
        srw = pr(); tt(out=srw[:], in0=s[:], in1=rcw[:], op=op.mult)
        rho = pr(); tt(out=rho[:], in0=srw[:], in1=srw[:], op=op.mult)
        grho4 = wk.tile([128, 4 * K], f32, name="grho4", tag="grho4")
        tt(out=grho4[:, 0:K], in0=gamma[:], in1=rho[:, 0:K], op=op.mult)
        tt(out=grho4[:, K:2 * K], in0=gamma[:], in1=rho[:, K:2 * K],
           op=op.mult)
        # shape-cost branch: -omiga = -|w1-w2|/max(w1,w2)
        wd = pr(); tt(out=wd[:], in0=wh1[:], in1=gtwh, op=op.subtract)
        wda = pr(); stt(out=wda[:], in0=wd[:], scalar=-1.0, in1=wd[:],
                        op0=op.mult, op1=op.max)
        mxw = pr(); tt(out=mxw[:], in0=wh1[:], in1=gtwh, op=op.max)
        rmx = pr(); nc.vector.reciprocal(out=rmx[:], in_=mxw[:])
        stt(out=grho4[:, 2 * K:4 * K], in0=wda[:], scalar=-1.0, in1=rmx[:],
            op0=op.mult, op1=op.mult)
        e4 = wk.tile([128, 4 * K], f32, name="e4", tag="e4")
        A_(out=e4[:, :], in_=grho4[:, :], func=act.Exp)
        oneo = pr(); ts(out=oneo[:], in0=e4[:, 2 * K:4 * K], scalar1=-1.0,
                        scalar2=1.0, op0=op.mult, op1=op.add)
        sq1 = pr(); tt(out=sq1[:], in0=oneo[:], in1=oneo[:], op=op.mult)
        sh = pr(); tt(out=sh[:], in0=sq1[:], in1=sq1[:], op=op.mult)
        egs = sg(); tt(out=egs[:], in0=e4[:, 0:K], in1=e4[:, K:2 * K],
                       op=op.add)
        dist = sg(); ts(out=dist[:], in0=egs[:], scalar1=-1.0, scalar2=2.0,
                        op0=op.mult, op1=op.add)
        shs = sg(); tt(out=shs[:], in0=lo(sh), in1=hi(sh), op=op.add)
        ds = sg(); tt(out=ds[:], in0=dist[:], in1=shs[:], op=op.add)
        siou = sg(); stt(out=siou[:], in0=ds[:], scalar=-0.5, in1=iou[:],
                         op0=op.mult, op1=op.add)
        onem = sg(); ts(out=onem[:], in0=siou[:], scalar1=-1.0, scalar2=1.0,
                        op0=op.mult, op1=op.add)
        jk1 = sg(); stt(out=jk1[:], in0=onem[:], scalar=1.0, in1=valid,
                        op0=op.mult, op1=op.mult,
                        accum_out=partials[:, 2 * NCH:2 * NCH + 1])
        # cls: one-hot select the gt class then ln
        msel = wk.tile([128, K, C], f32, name="msel", tag="msel")
        tt(out=msel[:, :, :], in0=gcls, in1=maskv, op=op.mult)
        csel = sg(); nc.vector.tensor_reduce(
            out=csel[:], in_=msel[:, :, :], axis=mybir.AxisListType.X,
            op=op.add)
        lnp = sg(); A_(out=lnp[:], in_=csel[:], func=act.Ln)
        jk2 = sg(); stt(out=jk2[:], in0=lnp[:], scalar=-1.0, in1=valid,
                        op0=op.mult, op1=op.mult,
                        accum_out=partials[:, 2 * NCH + 1:2 * NCH + 2])
        # obj corrections: sl1 via clamp identity; two win-weighted sums
        dif = sg(); tt(out=dif[:], in0=po, in1=siou[:], op=op.subtract)
        yc = sg(); ts(out=yc[:], in0=dif[:], scalar1=1.0, scalar2=-1.0,
                      op0=op.min, op1=op.max)
        zc = sg(); stt(out=zc[:], in0=yc[:], scalar=-0.5, in1=dif[:],
                       op0=op.mult, op1=op.add)
        sd = sg(); tt(out=sd[:], in0=yc[:], in1=zc[:], op=op.mult)
        t1 = sg(); stt(out=t1[:], in0=rc[:], scalar=float(0.25 * HW),
                       in1=sd[:], op0=op.mult, op1=op.mult)
        jk3 = sg(); stt(out=jk3[:], in0=t1[:], scalar=1.0, in1=win,
                        op0=op.mult, op1=op.mult,
                        accum_out=partials[:, 2 * NCH + 2:2 * NCH + 3])
        y2 = sg(); ts(out=y2[:], in0=po, scalar1=1.0, scalar2=-1.0,
                      op0=op.min, op1=op.max)
        z2 = sg(); stt(out=z2[:], in0=y2[:], scalar=-0.5, in1=po,
                       op0=op.mult, op1=op.add)
        s2m = sg(); stt(out=s2m[:], in0=y2[:], scalar=0.75, in1=z2[:],
                        op0=op.mult, op1=op.mult)
        jk4 = sg(); stt(out=jk4[:], in0=s2m[:], scalar=1.0, in1=win,
                        op0=op.mult, op1=op.mult,
                        accum_out=partials[:, 2 * NCH + 3:2 * NCH + 4])

        nc.sync.dma_start(out=out_p[:, :], in_=partials[:, :])

    return nc


def _get_nc(K, finalized=True):
    key = (K, finalized)
    if key not in _NC_CACHE:
        nc = _build_nc(K)
        if finalized:
            nc.finalize()
        else:
            nc.compile()
        _NC_CACHE[key] = nc
    return _NC_CACHE[key]


def _pack(vals, K, fill, dtype):
    """lane j = i*128 + p  ->  tile[p, i]."""
    out = np.full((K, 128), fill, dtype)
    out.reshape(-1)[:len(vals)] = vals
    return out.T


def host_prep(pred_obj, pred_delta_box, pred_cls, gt_box, gt_cls,
              p_batch_idx, p_x_idx, p_y_idx, p_anchor_idx, anchors):
    """Shard inputs; all-integer index prep. Returns (in_maps, K, P)."""
    import ml_dtypes
    f32 = np.float32
    pred_obj = np.asarray(pred_obj, f32)
    pred_delta_box = np.asarray(pred_delta_box, f32)
    pred_cls = np.asarray(pred_cls, f32)
    gt_box = np.asarray(gt_box, f32)
    gt_cls = np.asarray(gt_cls, np.int64)
    p_b = np.asarray(p_batch_idx, np.int64)
    p_x = np.asarray(p_x_idx, np.int64)
    p_y = np.asarray(p_y_idx, np.int64)
    p_a = np.asarray(p_anchor_idx, np.int64)
    anchors = np.asarray(anchors, f32)
    P = len(p_b)

    n_img = np.bincount(p_b, minlength=B)
    # duplicate (b,y,x,a) cells: last occurrence wins (matches XLA scatter)
    cell = ((p_b * H + p_y) * W + p_x) * A + p_a
    win = np.zeros(P, f32)
    _, ridx = np.unique(cell[::-1], return_index=True)
    win[P - 1 - ridx] = 1.0

    core_of = p_b // Bm
    counts = np.bincount(core_of, minlength=M)
    Pmax = int(counts.max())
    K = max(1, -(-Pmax // 128))

    in_maps = []
    for m in range(M):
        sel = core_of == m
        npos = int(sel.sum())
        bl = p_b[sel] - m * Bm
        xj, yj, aj, cj = p_x[sel], p_y[sel], p_a[sel], gt_cls[sel]
        base = bl * A + aj
        sp = yj * W + xj
        cell_idx = base * HW + sp
        off_rec = cell_idx * REC

        gtb = gt_box[sel]
        ancg = anchors[aj]
        hd_planes = [
            _pack(xj.astype(f32), K, 0.0, f32),
            _pack(yj.astype(f32), K, 0.0, f32),
            _pack(ancg[:, 0], K, 0.1, f32),
            _pack(ancg[:, 1], K, 0.1, f32),
            _pack(gtb[:, 0], K, 0.5, f32),
            _pack(gtb[:, 1], K, 0.5, f32),
            _pack(gtb[:, 2], K, 0.5, f32),
            _pack(gtb[:, 3], K, 0.5, f32),
            _pack(n_img[p_b[sel]].astype(f32), K, 1.0, f32),
            _pack(win[sel], K, 0.0, f32),
            _pack(np.ones(npos, f32), K, 0.0, f32),
        ]
        hd = np.concatenate(hd_planes, axis=1)  # [128, 11K] f32
        # one-hot class mask, record-major [i*20+c]; pads select class 0
        mask = np.zeros((K * 128, C), f32)
        mask[np.arange(npos), cj] = 1.0
        mask[npos:, 0] = 1.0
        mask = mask.reshape(K, 128, C).transpose(1, 0, 2).reshape(128, K * C)
        hdr = np.concatenate([
            _pack(off_rec, K, 0, np.int32),
            hd.view(np.int32),
            mask.view(np.int32),
        ], axis=1)

        obj_flat = pred_obj[m * Bm:(m + 1) * Bm].reshape(-1)
        rec = np.empty((NCELL, REC), f32)
        rec[:, 0:4] = pred_delta_box[m * Bm:(m + 1) * Bm] \
            .transpose(0, 1, 3, 4, 2).reshape(NCELL, 4)
        rec[:, 4] = obj_flat
        rec[:, 5:5 + C] = pred_cls[m * Bm:(m + 1) * Bm] \
            .transpose(0, 1, 3, 4, 2).reshape(NCELL, C)
        rec[:, 5 + C:] = 0.0

        in_maps.append({
            "rec": rec.reshape(NCELL * REC, 1),
            "objh": obj_flat.astype(ml_dtypes.bfloat16).reshape(SZ_OBJ, 1),
            "hdr": np.ascontiguousarray(hdr),
        })
    return in_maps, K, P


def combine(partials_list, P):
    """Host reduction of per-core [128, NCOLS] partial sums."""
    t_xy = t_yy = tot_iou = tot_cls = tot_t1w = tot_s2w = 0.0
    for pt in partials_list:
        pt = np.asarray(pt, np.float64)
        for c in range(NCH):
            t_xy += pt[:, 2 * c].sum()
            t_yy += pt[:, 2 * c + 1].sum()
        tot_iou += pt[:, 2 * NCH].sum()
        tot_cls += pt[:, 2 * NCH + 1].sum()
        tot_t1w += pt[:, 2 * NCH + 2].sum()
        tot_s2w += pt[:, 2 * NCH + 3].sum()
    iou_loss = tot_iou / P
    cls_loss = tot_cls / P
    tot_corr = tot_t1w - tot_s2w
    obj_loss = (0.75 * (t_xy - 0.5 * t_yy) + tot_corr) / (B * A * H * W)
    tot_loss = iou_loss + 4 * obj_loss + 2 * cls_loss
    return (np.float32(iou_loss), np.float32(obj_loss),
            np.float32(cls_loss), np.float32(tot_loss))


def kernel(pred_obj, pred_delta_box, pred_cls, gt_box, gt_cls,
           p_batch_idx, p_x_idx, p_y_idx, p_anchor_idx, anchors):
    from concourse.bass_utils import run_bass_kernel_spmd
    in_maps, K, P = host_prep(pred_obj, pred_delta_box, pred_cls, gt_box,
                              gt_cls, p_batch_idx, p_x_idx, p_y_idx,
                              p_anchor_idx, anchors)
    nc = _get_nc(K)
    res = run_bass_kernel_spmd(nc, in_maps, list(range(M))).results
    return combine([r["partials"] for r in res], P)
